# revision 11
# baseline (speedup 1.0000x reference)
"""Trainium2 Bass kernel for a fused autoregressive tanh-RNN decoder.

Model (per step t):
    h = tanh(x @ W_ih.T + b_ih + h @ W_hh.T + b_hh)   # h: [B,H], x: [B,1]
    y = h @ W_out.T + b_out                           # [B,1]
    x = tf[t] ? targets[t] : y
with T=256 steps, B=512, H=2048.

Sharding: data-parallel over batch — 64 rows per core on 8 cores; weights
replicated. The scan carry stays core-local so there is no per-step
communication.

Per-core kernel structure (fp16 matmul operands, fp32 PSUM accumulate):
  * The hidden state is kept TRANSPOSED (h^T, [H on partitions x 64 batch])
    and used as the matmul *stationary* operand; the big W_hh streams through
    the PE as the moving operand. This keeps LDWEIGHTS traffic tiny (64-col
    loads) and runs the PE at its streaming floor.
  * The 128x128 array is column-split into two halves (tile_position col 0 /
    col 64) that concurrently produce the two H/2 output halves.
  * x @ W_ih.T + (b_ih+b_hh) is folded into the same PSUM accumulation as an
    extra 2-row stationary pass ([x^T; ones] against [W_ih^T; bias]).
  * tanh on ScalarE (PSUM -> SBUF fp16), y = h.W_out via DVE
    tensor_tensor_reduce, teacher-force select via copy_predicated, and the
    next step's stationary h^T is rebuilt with PE transposes.
"""

import numpy as np

T, B, H = 256, 512, 2048
NCORES = 8
BC = B // NCORES          # 64 batch rows per core
JT = H // 128             # 16 contraction (K) tiles
HH = H // 2               # 1024, per-partition-half output columns

_CACHE = {}


def _build_program(n_steps):
    import concourse.bass as bass
    import concourse.tile as tile
    from concourse import bacc, mybir

    fp16 = mybir.dt.float16
    fp32 = mybir.dt.float32
    u8 = mybir.dt.uint8
    Tanh = mybir.ActivationFunctionType.Tanh
    Identity = mybir.ActivationFunctionType.Identity

    nc = bacc.Bacc("TRN2", target_bir_lowering=False, debug=False,
                   num_devices=NCORES)

    d_W = nc.dram_tensor("w_rhs", [128, JT * H], fp16, kind="ExternalInput")
    d_Wx = nc.dram_tensor("w_x", [2, H], fp16, kind="ExternalInput")
    d_Wout = nc.dram_tensor("w_out_rep", [128, JT], fp16, kind="ExternalInput")
    d_h0 = nc.dram_tensor("h0t", [128, JT * BC], fp16, kind="ExternalInput")
    d_x0 = nc.dram_tensor("x0t", [2, BC], fp16, kind="ExternalInput")
    d_tgt = nc.dram_tensor("tgt16", [BC, T], fp16, kind="ExternalInput")
    d_tf = nc.dram_tensor("tfmask", [BC, T], u8, kind="ExternalInput")
    d_bout = nc.dram_tensor("bout_rep", [BC, 1], fp32, kind="ExternalInput")
    d_id = nc.dram_tensor("ident", [128, BC], fp16, kind="ExternalInput")
    d_y = nc.dram_tensor("y_out", [BC, T], fp32, kind="ExternalOutput")

    with tile.TileContext(nc) as tc:
        with (
            tc.tile_pool(name="const", bufs=1) as constp,
            tc.tile_pool(name="stat", bufs=2) as statp,
            tc.tile_pool(name="hbuf", bufs=2) as hbufp,
            tc.tile_pool(name="scr", bufs=2) as scrp,
            tc.tile_pool(name="small", bufs=3) as smallp,
            tc.tile_pool(name="psmain", bufs=1, space="PSUM") as psmainp,
            tc.tile_pool(name="pstr", bufs=2, space="PSUM") as pstrp,
            tc.tile_pool(name="psx", bufs=2, space="PSUM") as psxp,
            tc.tile_pool(name="psy", bufs=2, space="PSUM") as psyp,
        ):
            # --- persistent SBUF residents -------------------------------
            sb_W = constp.tile([128, JT * H], fp16)
            nc.sync.dma_start(sb_W[:], d_W.ap())
            sb_Wx = constp.tile([2, H], fp16)
            nc.sync.dma_start(sb_Wx[:], d_Wx.ap())
            sb_WoutT = constp.tile([128, JT], fp16)
            nc.sync.dma_start(sb_WoutT[:], d_Wout.ap())
            sb_tgt = constp.tile([BC, T], fp16)
            nc.sync.dma_start(sb_tgt[:], d_tgt.ap())
            sb_tf = constp.tile([BC, T], u8)
            nc.sync.dma_start(sb_tf[:], d_tf.ap())
            sb_bout = constp.tile([BC, 1], fp32)
            nc.sync.dma_start(sb_bout[:], d_bout.ap())
            sb_id = constp.tile([128, BC], fp16)
            nc.sync.dma_start(sb_id[:], d_id.ap())
            sb_y = constp.tile([BC, T], fp32)
            nc.vector.memset(sb_y[:], 0.0)
            # [x^T; ones] stationary rows; row 0 is rewritten each step.
            sb_xstat = constp.tile([2, BC], fp16)
            nc.sync.dma_start(sb_xstat[:], d_x0.ap())

            stat = statp.tile([128, JT * BC], fp16)
            nc.sync.dma_start(stat[:], d_h0.ap())

            # j-tiles whose transposed sources live in the cp=0 PSUM bank
            # (free cols 0:512 of sb_h) come first so they can start as soon
            # as the first tanh fires.
            pair_order = (0, 1, 4, 5, 2, 3, 6, 7)

            for t in range(n_steps):
                ps = psmainp.tile([128, 2 * 512], fp32)
                sb_h = hbufp.tile([128, HH], fp16)

                for cp in (0, 1):
                    # main recurrent matmul, both column-halves concurrent
                    for j in range(JT):
                        lhsT = stat[:, j * BC:(j + 1) * BC]
                        for half in (0, 1):
                            nc.tensor.matmul(
                                ps[64 * half:64 * half + 64,
                                   cp * 512:(cp + 1) * 512],
                                lhsT,
                                sb_W[:, j * H + half * HH + cp * 512:
                                     j * H + half * HH + (cp + 1) * 512],
                                start=(j == 0), stop=False,
                                skip_group_check=True,
                            )
                    # x/bias rank-1 pass closes each accumulation group
                    for half in (0, 1):
                        nc.tensor.matmul(
                            ps[64 * half:64 * half + 64,
                               cp * 512:(cp + 1) * 512],
                            sb_xstat[:],
                            sb_Wx[:, half * HH + cp * 512:
                                  half * HH + (cp + 1) * 512],
                            start=False, stop=True,
                            skip_group_check=True,
                        )
                    nc.scalar.activation(
                        sb_h[:, cp * 512:(cp + 1) * 512],
                        ps[:, cp * 512:(cp + 1) * 512],
                        Tanh,
                    )

                # rebuild the transposed stationary for the next step
                statn = statp.tile([128, JT * BC], fp16)
                for pair in pair_order:
                    pt = pstrp.tile([128, 128], fp16)
                    for k2 in (0, 1):
                        j = 2 * pair + k2
                        half, blk = divmod(j, 8)
                        nc.tensor.transpose(
                            pt[:, 64 * k2:64 * k2 + 64],
                            sb_h[64 * half:64 * half + 64,
                                 blk * 128:(blk + 1) * 128],
                            sb_id[64 * half:64 * half + 64, 0:64],
                        )
                    nc.vector.tensor_copy(
                        statn[:, pair * 128:(pair + 1) * 128], pt[:])

                # y = h . W_out + b_out via PE matvec over the transposed
                # stationary (contraction over H on partitions), bias via ACT
                psy = psyp.tile([BC, 1], fp32, tag="ypart")
                for j in range(JT):
                    nc.tensor.matmul(
                        psy[:],
                        statn[:, j * BC:(j + 1) * BC],
                        sb_WoutT[:, j:j + 1],
                        start=(j == 0), stop=(j == JT - 1),
                    )
                nc.scalar.activation(
                    sb_y[:, t:t + 1], psy[:],
                    Identity, bias=sb_bout[:],
                )

                if t + 1 < n_steps:
                    # x' = tf ? target : y, cast to fp16, transpose to a row
                    x16 = smallp.tile([BC, 1], fp16, tag="x16")
                    nc.vector.tensor_copy(x16[:], sb_y[:, t:t + 1])
                    nc.vector.copy_predicated(
                        x16[:], sb_tf[:, t:t + 1], sb_tgt[:, t:t + 1])
                    px = psxp.tile([1, BC], fp16)
                    nc.tensor.transpose(px[:], x16[:], sb_id[0:64, 0:64])
                    nc.vector.tensor_copy(sb_xstat[0:1, :], px[:])

                stat = statn

            nc.sync.dma_start(d_y.ap(), sb_y[:])

    nc.compile()
    return nc


def _prep_inputs(initial_input, hidden, targets, W_ih, b_ih, W_hh, b_hh,
                 W_out, b_out, tf_mask):
    f16 = np.float16
    # moving operand: W[d, j*H + i] = W_hh[i, 128j+d]
    w = np.ascontiguousarray(W_hh.T.astype(f16))              # [j, i]
    w = w.reshape(JT, 128, H).transpose(1, 0, 2).reshape(128, JT * H)
    wx = np.stack([W_ih[:, 0], (b_ih + b_hh)]).astype(f16)    # [2, H]
    wout = np.ascontiguousarray(
        W_out[0].reshape(JT, 128).T).astype(f16)              # [128, JT]
    ident = np.concatenate([np.eye(BC), np.eye(BC)], axis=0).astype(f16)
    bout = np.full((BC, 1), np.float32(b_out[0]), np.float32)
    tf_u8 = np.tile(tf_mask.astype(np.uint8), (BC, 1))        # [BC, T]

    shared = dict(w_rhs=np.ascontiguousarray(w), w_x=np.ascontiguousarray(wx),
                  w_out_rep=np.ascontiguousarray(wout), ident=ident,
                  bout_rep=bout, tfmask=np.ascontiguousarray(tf_u8))

    in_maps = []
    for c in range(NCORES):
        s = slice(c * BC, (c + 1) * BC)
        h0 = hidden[s].astype(f16)                            # [BC, H]
        h0t = h0.T.reshape(JT, 128, BC).transpose(1, 0, 2).reshape(128, JT * BC)
        x0 = np.concatenate(
            [initial_input[s, 0][None, :], np.ones((1, BC))], axis=0
        ).astype(f16)                                         # [2, BC]
        tgt = np.ascontiguousarray(targets[:, s, 0].T).astype(f16)  # [BC, T]
        m = dict(shared)
        m.update(h0t=np.ascontiguousarray(h0t), x0t=x0,
                 tgt16=np.ascontiguousarray(tgt))
        in_maps.append(m)
    return in_maps


def kernel(initial_input, hidden, targets, W_ih, b_ih, W_hh, b_hh,
           W_out, b_out, tf_mask):
    from concourse.bass_utils import run_bass_kernel_spmd

    if "nc" not in _CACHE:
        _CACHE["nc"] = _build_program(T)
    nc = _CACHE["nc"]

    in_maps = _prep_inputs(initial_input, hidden, targets, W_ih, b_ih,
                           W_hh, b_hh, W_out, b_out, tf_mask)
    res = run_bass_kernel_spmd(nc, in_maps, list(range(NCORES)))
    # y_out per core: [BC, T] -> full output [T, B, 1]
    ys = [res.results[c]["y_out"] for c in range(NCORES)]
    out = np.concatenate([y.T[:, :, None] for y in ys], axis=1)
    return np.ascontiguousarray(out.astype(np.float32))
